# revision 1
# baseline (speedup 1.0000x reference)
"""Longformer sliding-window self-attention (B=2, S=4096, D=768, H=12, Dh=64,
one-sided window W=256) on 8 TRN2 NeuronCores.

Sharding: (batch, head-group) — core = b*4 + g handles batch b, heads
[3g, 3g+3). Each core runs the same SPMD Bass program on its shard:

  phase 1: X^T via PE transpose; Q^T/K^T/V^T = W^T @ X^T in float32r
           (TF32-like, ~1.6e-4 matmul relerr); V^T re-transposed into
           V_aug [s, 3*(64+1)] with a ones column per head (fused
           softmax-denominator).
  phase 2: per 256-query chunk and head, banded scores S^T[k, q] on PE
           (keys on partitions), exp on ACT straight out of PSUM (no
           max-subtraction -- scores for these input scales are far from
           overflow), band masking via two triangular 0/1 mask multiplies
           (only the 3 edge half-tiles per chunk need masking), then
           O^T = P^T.T @ V_aug accumulated over key tiles. The ones
           column yields Z; output rows are scaled by 1/Z on DVE.

kernel() takes full inputs, shards, runs SPMD on cores 0..7, reassembles.
"""
import sys

if '/opt/trn_rl_repo' not in sys.path:
    sys.path.insert(0, '/opt/trn_rl_repo')

import math
from contextlib import ExitStack

import numpy as np
import ml_dtypes

import concourse.bacc as bacc
import concourse.mybir as mybir
import concourse.tile as tile
from concourse.bass_utils import run_bass_kernel_spmd

F32 = mybir.dt.float32
F32R = mybir.dt.float32r
BF16 = mybir.dt.bfloat16

B, S, D = 2, 4096, 768
H, DH, W = 12, 64, 256
HPC = 3              # heads per core
DHC = HPC * DH       # 192 head-dims per core
NCORES = 8
C2 = 256             # query chunk
NCH = S // C2        # 16 chunks
NKT = S // 128       # 32 key tiles
SBLK = 512           # projection s-block
NSB = S // SBLK      # 8 s-blocks
VAW = DH + 1         # 65: V columns + ones column
AluOp = mybir.AluOpType
ActFn = mybir.ActivationFunctionType

# P / V_aug dtype for the attention-value matmul. BF16 is fast (1 cyc/row);
# F32 is the high-precision fallback (4 cyc/row).
AV_DT = BF16


def _build_program(use_fmask, use_qmask):
    nc = bacc.Bacc("TRN2", num_devices=NCORES)

    x_d = nc.dram_tensor("x", (S, D), F32, kind="ExternalInput").ap()
    wq_d = nc.dram_tensor("wq", (D, DHC), F32R, kind="ExternalInput").ap()
    wk_d = nc.dram_tensor("wk", (D, DHC), F32R, kind="ExternalInput").ap()
    wv_d = nc.dram_tensor("wv", (D, DHC), F32R, kind="ExternalInput").ap()
    bq_d = nc.dram_tensor("bq", (DHC, 1), F32, kind="ExternalInput").ap()
    bk_d = nc.dram_tensor("bk", (DHC, 1), F32, kind="ExternalInput").ap()
    bv_d = nc.dram_tensor("bv", (DHC, 1), F32, kind="ExternalInput").ap()
    id_d = nc.dram_tensor("ident", (128, 128), F32, kind="ExternalInput").ap()
    idlo_d = nc.dram_tensor("identlo", (128, 64), F32, kind="ExternalInput").ap()
    tge_d = nc.dram_tensor("t_ge", (128, 128), AV_DT, kind="ExternalInput").ap()
    tle_d = nc.dram_tensor("t_le", (128, 128), AV_DT, kind="ExternalInput").ap()
    if use_fmask:
        fmk_d = nc.dram_tensor("fmk", (128, NKT), F32, kind="ExternalInput").ap()
    if use_qmask:
        qmk_d = nc.dram_tensor("qmk", (128, NKT), F32, kind="ExternalInput").ap()
    out_d = nc.dram_tensor("out", (S, DHC), F32, kind="ExternalOutput").ap()

    with tile.TileContext(nc) as tc, ExitStack() as ctx:
        pers = ctx.enter_context(tc.tile_pool(name="pers", bufs=1))

        # persistent constants
        w_sb = {}
        b_sb = {}
        for nm, wd, bd in (("q", wq_d, bq_d), ("k", wk_d, bk_d), ("v", wv_d, bv_d)):
            wt = pers.tile([128, 6 * DHC], F32R, tag=f"w{nm}", name=f"w{nm}")
            nc.sync.dma_start(wt[:], wd.rearrange("(a p) n -> p a n", p=128))
            w_sb[nm] = wt
            bt0 = pers.tile([128, 1], F32, tag=f"b{nm}0", name=f"b{nm}0")
            bt1 = pers.tile([64, 1], F32, tag=f"b{nm}1", name=f"b{nm}1")
            nc.sync.dma_start(bt0[:], bd[0:128, :])
            nc.sync.dma_start(bt1[:], bd[128:DHC, :])
            b_sb[nm] = (bt0, bt1)
        ident = pers.tile([128, 128], F32, tag="ident", name="ident")
        identlo = pers.tile([128, 64], F32, tag="identlo", name="identlo")
        nc.sync.dma_start(ident[:], id_d)
        nc.sync.dma_start(identlo[:], idlo_d)
        t_ge = pers.tile([128, 128], AV_DT, tag="t_ge", name="t_ge")
        t_le = pers.tile([128, 128], AV_DT, tag="t_le", name="t_le")
        nc.sync.dma_start(t_ge[:], tge_d)
        nc.sync.dma_start(t_le[:], tle_d)
        if use_fmask:
            fmk = pers.tile([128, NKT], F32, tag="fmk", name="fmk")
            nc.sync.dma_start(fmk[:], fmk_d)
        if use_qmask:
            qmk = pers.tile([128, NKT], F32, tag="qmk", name="qmk")
            nc.sync.dma_start(qmk[:], qmk_d)

        # persistent activations: Q^T/K^T [dh, S] (f32r), V_aug [s, 32*195]
        qT0 = pers.tile([128, S], F32R, tag="qT0", name="qT0")
        qT1 = pers.tile([64, S], F32R, tag="qT1", name="qT1")
        kT0 = pers.tile([128, S], F32R, tag="kT0", name="kT0")
        kT1 = pers.tile([64, S], F32R, tag="kT1", name="kT1")
        vT0 = pers.tile([128, S], F32, tag="vT0", name="vT0")
        vT1 = pers.tile([64, S], F32, tag="vT1", name="vT1")
        va = pers.tile([128, NKT * HPC * VAW], AV_DT, tag="va", name="va")
        va4 = va.rearrange("p (t h c) -> p t h c", h=HPC, c=VAW)
        nc.gpsimd.memset(va4[:, :, :, DH:VAW], 1.0)

        outT = {"q": (qT0, qT1), "k": (kT0, kT1), "v": (vT0, vT1)}

        # ---------------- phase 1: X^T, projections, V_aug ----------------
        with tc.tile_pool(name="p1s", bufs=2) as p1s, \
             tc.tile_pool(name="pp_tp", bufs=4, space="PSUM") as pp_tp, \
             tc.tile_pool(name="pp_pj", bufs=2, space="PSUM") as pp_pj, \
             tc.tile_pool(name="pp_tv", bufs=2, space="PSUM") as pp_tv:
            for sb in range(NSB):
                xin = p1s.tile([128, 4 * D], F32, tag="xin", name="xin", bufs=2)
                xin3 = xin.rearrange("p (a d) -> p a d", a=4)
                nc.sync.dma_start(
                    xin3[:], x_d[sb * SBLK:(sb + 1) * SBLK, :]
                    .rearrange("(a p) d -> p a d", p=128))
                xt = [p1s.tile([128, SBLK], F32R, tag=f"xt{dt}", name=f"xt{dt}", bufs=2)
                      for dt in range(6)]
                for dt in range(6):
                    tp = pp_tp.tile([128, SBLK], F32, tag="tp", name="tp")
                    for st in range(4):
                        nc.tensor.transpose(
                            tp[:, st * 128:(st + 1) * 128],
                            xin3[:, st, dt * 128:(dt + 1) * 128], ident[:])
                    nc.vector.tensor_copy(xt[dt][:], tp[:])
                for nm in ("q", "k", "v"):
                    wt = w_sb[nm]
                    for mt, (m0, msz) in enumerate(((0, 128), (128, 64))):
                        ps = pp_pj.tile([msz, SBLK], F32, tag="pj", name="pj")
                        for kt in range(6):
                            nc.tensor.matmul(
                                ps[:],
                                wt[:, kt * DHC + m0: kt * DHC + m0 + msz],
                                xt[kt][:],
                                start=(kt == 0), stop=(kt == 5))
                        dst = outT[nm][mt][:, sb * SBLK:(sb + 1) * SBLK]
                        nc.vector.tensor_scalar_add(dst, ps[:], b_sb[nm][mt][:])
                # V_aug for this block's 4 s-tiles, grouped per head
                for h in range(HPC):
                    tv = pp_tv.tile([128, 4 * DH], F32, tag="tv", name="tv")
                    for st in range(4):
                        gst = sb * 4 + st
                        if h == 0:
                            src = vT0[0:64, gst * 128:(gst + 1) * 128]
                            idn = ident[0:64, 0:64]
                        elif h == 1:
                            src = vT0[64:128, gst * 128:(gst + 1) * 128]
                            idn = identlo[64:128, :]
                        else:
                            src = vT1[0:64, gst * 128:(gst + 1) * 128]
                            idn = ident[0:64, 0:64]
                        nc.tensor.transpose(
                            tv[:, st * DH:(st + 1) * DH], src, idn)
                    nc.vector.tensor_copy(
                        va4[:, sb * 4:(sb + 1) * 4, h, 0:DH],
                        tv.rearrange("p (a d) -> p a d", a=4))

        # ---------------- phase 2: banded attention ----------------
        with tc.tile_pool(name="p2s", bufs=1) as p2s, \
             tc.tile_pool(name="pp_sc", bufs=4, space="PSUM") as pp_sc, \
             tc.tile_pool(name="pp_av", bufs=4, space="PSUM") as pp_av:
            for ci in range(NCH):
                os_t = [p2s.tile([128, DHC], F32, tag="os", name="os", bufs=4)
                        for _ in range(2)]
                av_big = pp_av.tile([128, 6 * VAW], F32, tag="av", name="av",
                                    bufs=2)
                av6 = av_big.rearrange("p (g c) -> p g c", c=VAW)
                for h in range(HPC):
                    if h < 2:
                        r0 = h * 64
                        qS, kS = qT0, kT0
                    else:
                        r0 = 0
                        qS, kS = qT1, kT1
                    kt0 = max(0, 2 * ci - 2)
                    kt1 = min(NKT - 1, 2 * ci + 3)
                    nkt = kt1 - kt0 + 1
                    pts = {0: [], 1: []}   # half -> [(kt, pt_slice)]
                    sc = pp_sc.tile([128, 6 * C2], F32, tag="sc", name="sc",
                                    bufs=2)
                    for kt in range(kt0, kt1 + 1):
                        i = kt - kt0
                        nc.tensor.matmul(
                            sc[:, i * C2:(i + 1) * C2],
                            kS[r0:r0 + 64, kt * 128:(kt + 1) * 128],
                            qS[r0:r0 + 64, ci * C2:(ci + 1) * C2],
                            start=True, stop=True)
                    pt = p2s.tile([128, 6 * C2], AV_DT, tag="pt", name="pt",
                                  bufs=3)
                    nc.scalar.activation(pt[:, 0:nkt * C2], sc[:, 0:nkt * C2],
                                         ActFn.Exp)
                    for kt in range(kt0, kt1 + 1):
                        j = kt - 2 * ci
                        i = kt - kt0
                        p0 = pt[:, i * C2:i * C2 + 128]
                        p1 = pt[:, i * C2 + 128:(i + 1) * C2]
                        if j == -2:
                            nc.gpsimd.tensor_tensor(p0, p0, t_ge[:], op=AluOp.mult)
                        elif j == -1:
                            nc.gpsimd.tensor_tensor(p1, p1, t_ge[:], op=AluOp.mult)
                        elif j == 2:
                            nc.gpsimd.tensor_tensor(p0, p0, t_le[:], op=AluOp.mult)
                        elif j == 3:
                            nc.gpsimd.tensor_tensor(p1, p1, t_le[:], op=AluOp.mult)
                        if use_fmask:
                            nc.vector.tensor_scalar_mul(
                                pt[:, i * C2:(i + 1) * C2],
                                pt[:, i * C2:(i + 1) * C2], fmk[:, kt:kt + 1])
                        if j != 3:
                            pts[0].append((kt, p0))
                        if j != -2:
                            pts[1].append((kt, p1))
                    for hf in range(2):
                        lst = pts[hf]
                        g = h * 2 + hf
                        for i, (kt, psl) in enumerate(lst):
                            nc.tensor.matmul(
                                av6[:, g, :], psl, va4[:, kt, h, :],
                                start=(i == 0), stop=(i == len(lst) - 1))
                # epilogue: one reciprocal over the 6 Z columns, then 6
                # scaled copies on ACT
                rzs = p2s.tile([128, 6], F32, tag="rzs", name="rzs", bufs=3)
                nc.vector.reciprocal(rzs[:], av6[:, :, DH])
                if use_qmask:
                    for g in range(6):
                        nc.vector.tensor_scalar_mul(
                            rzs[:, g:g + 1], rzs[:, g:g + 1],
                            qmk[:, 2 * ci + (g % 2):2 * ci + (g % 2) + 1])
                for h in range(HPC):
                    for hf in range(2):
                        g = h * 2 + hf
                        nc.scalar.activation(
                            os_t[hf][:, h * DH:(h + 1) * DH], av6[:, g, 0:DH],
                            ActFn.Copy, scale=rzs[:, g:g + 1])
                for hf in range(2):
                    qt = 2 * ci + hf
                    nc.sync.dma_start(
                        out_d[qt * 128:(qt + 1) * 128, :], os_t[hf][:])

    nc.compile()
    return nc


_prog_cache = {}


def _get_program(use_fmask, use_qmask):
    key = (use_fmask, use_qmask)
    if key not in _prog_cache:
        _prog_cache[key] = _build_program(use_fmask, use_qmask)
    return _prog_cache[key]


def _host_constants():
    kl = np.arange(128)[:, None]
    ql = np.arange(128)[None, :]
    np_av = mybir.dt.np(AV_DT)
    t_ge = (kl >= ql).astype(np_av)
    t_le = (kl <= ql).astype(np_av)
    ident = np.eye(128, dtype=np.float32)
    identlo = np.zeros((128, 64), dtype=np.float32)
    identlo[64:128, :] = np.eye(64, dtype=np.float32)
    return ident, identlo, t_ge, t_le


def kernel(hidden_states, attention_mask, is_index_masked, Wq, bq, Wk, bk, Wv, bv,
           trace=False):
    hidden_states = np.asarray(hidden_states, dtype=np.float32)
    attention_mask = np.asarray(attention_mask, dtype=np.float32)
    is_index_masked = np.asarray(is_index_masked)
    Wq = np.asarray(Wq, dtype=np.float32)
    Wk = np.asarray(Wk, dtype=np.float32)
    Wv = np.asarray(Wv, dtype=np.float32)
    bq = np.asarray(bq, dtype=np.float32)
    bk = np.asarray(bk, dtype=np.float32)
    bv = np.asarray(bv, dtype=np.float32)

    use_fmask = bool(np.any(attention_mask != 0))
    use_qmask = bool(np.any(is_index_masked))
    nc = _get_program(use_fmask, use_qmask)

    scale = 1.0 / math.sqrt(DH)
    ident, identlo, t_ge, t_le = _host_constants()

    in_maps = []
    for cid in range(NCORES):
        b = cid // 4
        h0 = HPC * (cid % 4)
        c0, c1 = h0 * DH, (h0 + HPC) * DH
        m = {
            "x": hidden_states[b],
            "wq": np.ascontiguousarray(Wq[:, c0:c1] * scale),
            "wk": np.ascontiguousarray(Wk[:, c0:c1]),
            "wv": np.ascontiguousarray(Wv[:, c0:c1]),
            "bq": np.ascontiguousarray((bq[c0:c1] * scale).reshape(DHC, 1)),
            "bk": np.ascontiguousarray(bk[c0:c1].reshape(DHC, 1)),
            "bv": np.ascontiguousarray(bv[c0:c1].reshape(DHC, 1)),
            "ident": ident,
            "identlo": identlo,
            "t_ge": t_ge,
            "t_le": t_le,
        }
        if use_fmask:
            fac = (attention_mask[b] == 0).astype(np.float32)  # keep-factor
            m["fmk"] = np.ascontiguousarray(fac.reshape(NKT, 128).T)
        if use_qmask:
            keep = (~is_index_masked[b]).astype(np.float32)
            m["qmk"] = np.ascontiguousarray(keep.reshape(NKT, 128).T)
        in_maps.append(m)

    res = run_bass_kernel_spmd(nc, in_maps, core_ids=list(range(NCORES)),
                               trace=trace)
    out = np.empty((B, S, D), dtype=np.float32)
    for cid in range(NCORES):
        b = cid // 4
        h0 = HPC * (cid % 4)
        out[b, :, h0 * DH:(h0 + HPC) * DH] = res.results[cid]["out"]
    if trace:
        return out, res
    return out



# revision 4
# speedup vs baseline: 1.4471x; 1.4471x over previous
"""Longformer sliding-window self-attention (B=2, S=4096, D=768, H=12, Dh=64,
one-sided window W=256) on 8 TRN2 NeuronCores.

Sharding: (batch, head-group) — core = b*4 + g handles batch b, heads
[3g, 3g+3). All-bf16 operand path (f32 PSUM accumulation):

  A: X^T via PE transpose in bf16 (host pre-casts X to bf16).
  B: fused Q/K projection W_qk^T @ X^T with weight-stationary loop
     (one LDWEIGHTS per 8 matmuls); PSUM->SBUF casts on ACT.
  C: V computed directly row-major (X^T tiles stationary, Wv moving)
     into V_aug [s, kt, h, 65] with a ones column (softmax denominator).
  D: per 256-query chunk: banded scores S^T[k, q] with h0/h1 issued as
     row-tiled pairs (K=64 at base partitions 0/64 -> concurrent on PE),
     exp on ACT out of PSUM, band-edge masking via t_ge/t_le multiplies
     on DVE (bf16 2x mode), AV accumulation with ones-column Z, output
     rows scaled by 1/Z on GpSimd.

kernel() takes full inputs, shards, runs SPMD on cores 0..7, reassembles.
"""
import sys

if '/opt/trn_rl_repo' not in sys.path:
    sys.path.insert(0, '/opt/trn_rl_repo')

import math
from contextlib import ExitStack

import numpy as np
import ml_dtypes

import concourse.bacc as bacc
import concourse.mybir as mybir
import concourse.tile as tile
from concourse.bass_utils import run_bass_kernel_spmd

F32 = mybir.dt.float32
BF16 = mybir.dt.bfloat16

B, S, D = 2, 4096, 768
H, DH, W = 12, 64, 256
HPC = 3              # heads per core
DHC = HPC * DH       # 192 head-dims per core
NCORES = 8
C2 = 256             # query chunk
NCH = S // C2        # 16 chunks
NKT = S // 128       # 32 key tiles
SBLK = 512           # projection s-block
NSB = S // SBLK      # 8 s-blocks
VAW = DH + 1         # 65: V columns + ones column
MQK = 2 * DHC        # 384 fused q+k output dims
AluOp = mybir.AluOpType
ActFn = mybir.ActivationFunctionType
NPBF16 = ml_dtypes.bfloat16


def _build_program(use_b, use_bv, use_fmask, use_qmask):
    nc = bacc.Bacc("TRN2", num_devices=NCORES)

    xb_d = nc.dram_tensor("xb", (S, D), BF16, kind="ExternalInput").ap()
    wqk_d = nc.dram_tensor("wqk", (D, MQK), BF16, kind="ExternalInput").ap()
    wv_d = nc.dram_tensor("wv", (D, DHC), BF16, kind="ExternalInput").ap()
    idb_d = nc.dram_tensor("identb", (128, 128), BF16, kind="ExternalInput").ap()
    tge_d = nc.dram_tensor("t_ge", (128, 128), BF16, kind="ExternalInput").ap()
    tle_d = nc.dram_tensor("t_le", (128, 128), BF16, kind="ExternalInput").ap()
    if use_b:
        bqk_d = nc.dram_tensor("bqk", (MQK, 1), F32, kind="ExternalInput").ap()
    if use_bv:
        bvr_d = nc.dram_tensor("bvr", (1, DHC), BF16, kind="ExternalInput").ap()
    if use_fmask:
        fmk_d = nc.dram_tensor("fmk", (128, NKT), F32, kind="ExternalInput").ap()
    if use_qmask:
        qmk_d = nc.dram_tensor("qmk", (128, NKT), F32, kind="ExternalInput").ap()
    out_d = nc.dram_tensor("out", (S, DHC), F32, kind="ExternalOutput").ap()

    with tile.TileContext(nc) as tc, ExitStack() as ctx:
        pers = ctx.enter_context(tc.tile_pool(name="pers", bufs=1))

        wqk = pers.tile([128, 6 * MQK], BF16, tag="wqk", name="wqk")
        nc.sync.dma_start(wqk[:], wqk_d.rearrange("(a p) n -> p a n", p=128))
        wqk3 = wqk.rearrange("p (a n) -> p a n", a=6)
        wv = pers.tile([128, 6 * DHC], BF16, tag="wv", name="wv")
        nc.sync.dma_start(wv[:], wv_d.rearrange("(a p) n -> p a n", p=128))
        wv3 = wv.rearrange("p (a n) -> p a n", a=6)
        identb = pers.tile([128, 128], BF16, tag="identb", name="identb")
        nc.sync.dma_start(identb[:], idb_d)
        t_ge = pers.tile([128, 128], BF16, tag="t_ge", name="t_ge")
        t_le = pers.tile([128, 128], BF16, tag="t_le", name="t_le")
        nc.sync.dma_start(t_ge[:], tge_d)
        nc.sync.dma_start(t_le[:], tle_d)
        if use_b:
            bqk = pers.tile([128, 3], F32, tag="bqk", name="bqk")
            nc.sync.dma_start(bqk[:], bqk_d.rearrange("(m p) c -> p (m c)", p=128))
        if use_bv:
            bvr = pers.tile([1, DHC], BF16, tag="bvr", name="bvr")
            nc.sync.dma_start(bvr[:], bvr_d)
        if use_fmask:
            fmk = pers.tile([128, NKT], F32, tag="fmk", name="fmk")
            nc.sync.dma_start(fmk[:], fmk_d)
        if use_qmask:
            qmk = pers.tile([128, NKT], F32, tag="qmk", name="qmk")
            nc.sync.dma_start(qmk[:], qmk_d)

        # persistent activations
        xT = pers.tile([128, 6 * S], BF16, tag="xT", name="xT")
        xT3 = xT.rearrange("p (a s) -> p a s", a=6)
        qT01 = pers.tile([128, S], BF16, tag="qT01", name="qT01")
        kT01 = pers.tile([128, S], BF16, tag="kT01", name="kT01")
        qT2 = pers.tile([64, S], BF16, tag="qT2", name="qT2")
        kT2 = pers.tile([64, S], BF16, tag="kT2", name="kT2")
        va = pers.tile([128, NKT * HPC * VAW], BF16, tag="va", name="va")
        va4 = va.rearrange("p (t h c) -> p t h c", h=HPC, c=VAW)
        nc.gpsimd.memset(va4[:, :, :, DH:VAW], 1.0)
        # pre-warm the exp table while phase A runs
        scr = pers.tile([1, 1], BF16, tag="scr", name="scr")
        nc.scalar.activation(scr[:], identb[0:1, 0:1], ActFn.Exp)

        # ---------------- phase A: X^T (bf16 PE transposes) ----------------
        with tc.tile_pool(name="pA", bufs=2) as pA, \
             tc.tile_pool(name="psA", bufs=4, space="PSUM") as psA:
            for sb in range(NSB):
                xin = pA.tile([128, 4 * D], BF16, tag="xin", name="xin", bufs=2)
                xin3 = xin.rearrange("p (a d) -> p a d", a=4)
                nc.sync.dma_start(
                    xin3[:], xb_d[sb * SBLK:(sb + 1) * SBLK, :]
                    .rearrange("(a p) d -> p a d", p=128))
                for dt in range(6):
                    tp = psA.tile([128, SBLK], BF16, tag="tp", name="tp")
                    for st in range(4):
                        nc.tensor.transpose(
                            tp[:, st * 128:(st + 1) * 128],
                            xin3[:, st, dt * 128:(dt + 1) * 128], identb[:])
                    nc.vector.tensor_copy(
                        xT3[:, dt, sb * SBLK:(sb + 1) * SBLK], tp[:])

        # ---------------- phase B: fused Q/K projection ----------------
        # M layout: m0 = q dims 0..127 (h0,h1), m1 = k dims 0..127 (h0,h1),
        #           m2 = q dims 128..191 (h2) | k dims 128..191 (h2)
        with tc.tile_pool(name="psB", bufs=8, space="PSUM") as psB:
            for m in range(3):
                ps = [psB.tile([128, SBLK], F32, tag="pb", name=f"pb{m}_{sb}",
                               bufs=8) for sb in range(NSB)]
                for k in range(6):
                    for sb in range(NSB):
                        nc.tensor.matmul(
                            ps[sb][:],
                            wqk3[:, k, m * 128:(m + 1) * 128],
                            xT3[:, k, sb * SBLK:(sb + 1) * SBLK],
                            start=(k == 0), stop=(k == 5))
                for sb in range(NSB):
                    sl = slice(sb * SBLK, (sb + 1) * SBLK)
                    if m == 0:
                        dsts = [(qT01[:, sl], ps[sb][:], None)]
                    elif m == 1:
                        dsts = [(kT01[:, sl], ps[sb][:], None)]
                    else:
                        dsts = [(qT2[:, sl], ps[sb][0:64, :], (0, 64)),
                                (kT2[:, sl], ps[sb][64:128, :], (64, 128))]
                    for dst, src, rows in dsts:
                        if use_b:
                            if rows is None:
                                bias = bqk[:, m:m + 1]
                            else:
                                bias = bqk[rows[0]:rows[1], m:m + 1]
                            nc.vector.tensor_scalar_add(dst, src, bias)
                        else:
                            nc.scalar.activation(dst, src, ActFn.Copy)

        # ---------------- phase C: V row-major into V_aug ----------------
        with tc.tile_pool(name="psC", bufs=4, space="PSUM") as psC:
            for st in range(NKT):
                vps = psC.tile([128, DHC], F32, tag="vps", name="vps", bufs=4)
                for k in range(6):
                    nc.tensor.matmul(
                        vps[:],
                        xT3[:, k, st * 128:(st + 1) * 128],
                        wv3[:, k, :],
                        start=(k == 0), stop=(k == 5 and not use_bv))
                if use_bv:
                    # bias via K=1 ones-row matmul (t_ge row 127 is all ones)
                    nc.tensor.matmul(
                        vps[:], t_ge[127:128, 0:128], bvr[0:1, :],
                        start=False, stop=True)
                nc.scalar.activation(
                    va4[:, st, :, 0:DH],
                    vps.rearrange("p (h d) -> p h d", h=HPC), ActFn.Copy)

        # ---------------- phase D: banded attention ----------------
        with tc.tile_pool(name="pD", bufs=1) as pD, \
             tc.tile_pool(name="psD_sc", bufs=2, space="PSUM") as psD_sc, \
             tc.tile_pool(name="psD_av", bufs=2, space="PSUM") as psD_av:
            for ci in range(NCH):
                kt0 = max(0, 2 * ci - 2)
                kt1 = min(NKT - 1, 2 * ci + 3)
                nkt = kt1 - kt0 + 1
                av_big = psD_av.tile([128, 6 * VAW], F32, tag="av", name="av",
                                     bufs=2)
                av6 = av_big.rearrange("p (g c) -> p g c", c=VAW)
                os_t = [pD.tile([128, DHC], F32, tag="os", name="os", bufs=4)
                        for _ in range(2)]
                pts_all = []
                # h0/h1: row-tiled pairs (base partitions 0 / 64)
                sc0 = psD_sc.tile([128, 6 * C2], F32, tag="sc", name="sc0",
                                  bufs=2)
                sc1 = psD_sc.tile([128, 6 * C2], F32, tag="sc", name="sc1",
                                  bufs=2)
                qsl = slice(ci * C2, (ci + 1) * C2)
                for kt in range(kt0, kt1 + 1):
                    i = kt - kt0
                    ksl = slice(kt * 128, (kt + 1) * 128)
                    osl = slice(i * C2, (i + 1) * C2)
                    nc.tensor.matmul(sc0[:, osl], kT01[0:64, ksl],
                                     qT01[0:64, qsl], start=True, stop=True)
                    nc.tensor.matmul(sc1[:, osl], kT01[64:128, ksl],
                                     qT01[64:128, qsl], start=True, stop=True)
                pt0 = pD.tile([128, 6 * C2], BF16, tag="pt", name="pt0", bufs=3)
                pt1 = pD.tile([128, 6 * C2], BF16, tag="pt", name="pt1", bufs=3)
                nc.scalar.activation(pt0[:, 0:nkt * C2], sc0[:, 0:nkt * C2],
                                     ActFn.Exp)
                nc.scalar.activation(pt1[:, 0:nkt * C2], sc1[:, 0:nkt * C2],
                                     ActFn.Exp)
                # h2: solo (waits on sc pool buffer rotation)
                sc2 = psD_sc.tile([128, 6 * C2], F32, tag="sc", name="sc2",
                                  bufs=2)
                for kt in range(kt0, kt1 + 1):
                    i = kt - kt0
                    ksl = slice(kt * 128, (kt + 1) * 128)
                    nc.tensor.matmul(sc2[:, i * C2:(i + 1) * C2],
                                     kT2[0:64, ksl], qT2[0:64, qsl],
                                     start=True, stop=True)
                pt2 = pD.tile([128, 6 * C2], BF16, tag="pt", name="pt2", bufs=3)
                nc.scalar.activation(pt2[:, 0:nkt * C2], sc2[:, 0:nkt * C2],
                                     ActFn.Exp)
                # band-edge masks on DVE + AV accumulation
                for h, pt in enumerate((pt0, pt1, pt2)):
                    pts = {0: [], 1: []}
                    for kt in range(kt0, kt1 + 1):
                        j = kt - 2 * ci
                        i = kt - kt0
                        p0 = pt[:, i * C2:i * C2 + 128]
                        p1 = pt[:, i * C2 + 128:(i + 1) * C2]
                        if j == -2:
                            nc.vector.tensor_tensor(p0, p0, t_ge[:],
                                                    op=AluOp.mult)
                        elif j == -1:
                            nc.vector.tensor_tensor(p1, p1, t_ge[:],
                                                    op=AluOp.mult)
                        elif j == 2:
                            nc.vector.tensor_tensor(p0, p0, t_le[:],
                                                    op=AluOp.mult)
                        elif j == 3:
                            nc.vector.tensor_tensor(p1, p1, t_le[:],
                                                    op=AluOp.mult)
                        if use_fmask:
                            nc.vector.tensor_scalar_mul(
                                pt[:, i * C2:(i + 1) * C2],
                                pt[:, i * C2:(i + 1) * C2], fmk[:, kt:kt + 1])
                        if j != 3:
                            pts[0].append((kt, p0))
                        if j != -2:
                            pts[1].append((kt, p1))
                    for hf in range(2):
                        lst = pts[hf]
                        g = h * 2 + hf
                        for i, (kt, psl) in enumerate(lst):
                            nc.tensor.matmul(
                                av6[:, g, :], psl, va4[:, kt, h, :],
                                start=(i == 0), stop=(i == len(lst) - 1))
                # epilogue: 1/Z then scaled copies on GpSimd
                rzs = pD.tile([128, 6], F32, tag="rzs", name="rzs", bufs=3)
                nc.vector.reciprocal(rzs[:], av6[:, :, DH])
                if use_qmask:
                    for g in range(6):
                        nc.vector.tensor_scalar_mul(
                            rzs[:, g:g + 1], rzs[:, g:g + 1],
                            qmk[:, 2 * ci + (g % 2):2 * ci + (g % 2) + 1])
                for h in range(HPC):
                    for hf in range(2):
                        g = h * 2 + hf
                        nc.vector.tensor_scalar_mul(
                            os_t[hf][:, h * DH:(h + 1) * DH],
                            av6[:, g, 0:DH], rzs[:, g:g + 1])
                for hf in range(2):
                    qt = 2 * ci + hf
                    nc.sync.dma_start(
                        out_d[qt * 128:(qt + 1) * 128, :], os_t[hf][:])

    nc.compile()
    return nc


_prog_cache = {}


def _get_program(use_b, use_bv, use_fmask, use_qmask):
    key = (use_b, use_bv, use_fmask, use_qmask)
    if key not in _prog_cache:
        _prog_cache[key] = _build_program(use_b, use_bv, use_fmask, use_qmask)
    return _prog_cache[key]


def _host_constants():
    kl = np.arange(128)[:, None]
    ql = np.arange(128)[None, :]
    t_ge = (kl >= ql).astype(NPBF16)
    t_le = (kl <= ql).astype(NPBF16)
    identb = np.eye(128, dtype=np.float32).astype(NPBF16)
    return identb, t_ge, t_le


def kernel(hidden_states, attention_mask, is_index_masked, Wq, bq, Wk, bk, Wv, bv,
           trace=False):
    hidden_states = np.asarray(hidden_states, dtype=np.float32)
    attention_mask = np.asarray(attention_mask, dtype=np.float32)
    is_index_masked = np.asarray(is_index_masked)
    Wq = np.asarray(Wq, dtype=np.float32)
    Wk = np.asarray(Wk, dtype=np.float32)
    Wv = np.asarray(Wv, dtype=np.float32)
    bq = np.asarray(bq, dtype=np.float32)
    bk = np.asarray(bk, dtype=np.float32)
    bv = np.asarray(bv, dtype=np.float32)

    use_b = bool(np.any(bq != 0) or np.any(bk != 0))
    use_bv = bool(np.any(bv != 0))
    use_fmask = bool(np.any(attention_mask != 0))
    use_qmask = bool(np.any(is_index_masked))
    nc = _get_program(use_b, use_bv, use_fmask, use_qmask)

    scale = 1.0 / math.sqrt(DH)
    identb, t_ge, t_le = _host_constants()

    xb = hidden_states.astype(NPBF16)

    in_maps = []
    for cid in range(NCORES):
        b = cid // 4
        h0 = HPC * (cid % 4)
        c0, c1 = h0 * DH, (h0 + HPC) * DH
        wql = Wq[:, c0:c1] * scale
        wkl = Wk[:, c0:c1]
        wqk_h = np.concatenate(
            [wql[:, 0:128], wkl[:, 0:128], wql[:, 128:192], wkl[:, 128:192]],
            axis=1).astype(NPBF16)
        m = {
            "xb": xb[b],
            "wqk": np.ascontiguousarray(wqk_h),
            "wv": np.ascontiguousarray(Wv[:, c0:c1].astype(NPBF16)),
            "identb": identb,
            "t_ge": t_ge,
            "t_le": t_le,
        }
        if use_b:
            bql = bq[c0:c1] * scale
            bkl = bk[c0:c1]
            m["bqk"] = np.ascontiguousarray(np.concatenate(
                [bql[0:128], bkl[0:128], bql[128:192], bkl[128:192]])
                .reshape(MQK, 1))
        if use_bv:
            m["bvr"] = np.ascontiguousarray(
                bv[c0:c1].astype(NPBF16).reshape(1, DHC))
        if use_fmask:
            fac = (attention_mask[b] == 0).astype(np.float32)
            m["fmk"] = np.ascontiguousarray(fac.reshape(NKT, 128).T)
        if use_qmask:
            keep = (~is_index_masked[b]).astype(np.float32)
            m["qmk"] = np.ascontiguousarray(keep.reshape(NKT, 128).T)
        in_maps.append(m)

    res = run_bass_kernel_spmd(nc, in_maps, core_ids=list(range(NCORES)),
                               trace=trace)
    out = np.empty((B, S, D), dtype=np.float32)
    for cid in range(NCORES):
        b = cid // 4
        h0 = HPC * (cid % 4)
        out[b, :, h0 * DH:(h0 + HPC) * DH] = res.results[cid]["out"]
    if trace:
        return out, res
    return out


# revision 5
# speedup vs baseline: 1.6567x; 1.1449x over previous
"""Longformer sliding-window self-attention (B=2, S=4096, D=768, H=12, Dh=64,
one-sided window W=256) on 8 TRN2 NeuronCores.

Sharding: (batch, head-group) — core = b*4 + g handles batch b, heads
[3g, 3g+3). All-bf16 operand path (f32 PSUM accumulation):

  phase 1 (per 512-row s-block, pipelined):
    X^T loaded directly via DMA xbar transpose (host pre-casts X to bf16),
    fused Q/K projection W_qk^T @ X^T (bf16, full 128-row m-tiles),
    V computed row-major (X^T tiles stationary, Wv moving) into V_aug
    [s, kt, h, 65] with a ones column (softmax denominator). All
    PSUM->SBUF copies on DVE.
  phase 2 (per 256-query chunk, lag-2 software pipeline):
    folded-edge banded scores S^T[k, q]: 4 full key tiles + the two edge
    half-tiles packed into one 256-col slot (slot 0), h0/h1 issued as
    row-tiled pairs (K=64 at base partitions 0/64), exp on ACT straight
    out of PSUM, band-edge masking via t_ge/t_le multiplies on DVE (bf16
    2x mode), AV accumulation with ones-column Z, output rows scaled by
    1/Z on DVE. Emission order: scores/exp(ci), masks/AV(ci-1),
    recip/epilogue/store(ci-2) — keeps every engine queue stall-free.

kernel() takes full inputs, shards, runs SPMD on cores 0..7, reassembles.
"""
import sys

if '/opt/trn_rl_repo' not in sys.path:
    sys.path.insert(0, '/opt/trn_rl_repo')

import math
from contextlib import ExitStack

import numpy as np
import ml_dtypes

import concourse.bacc as bacc
import concourse.mybir as mybir
import concourse.tile as tile
from concourse.bass_utils import run_bass_kernel_spmd

F32 = mybir.dt.float32
BF16 = mybir.dt.bfloat16

B, S, D = 2, 4096, 768
H, DH, W = 12, 64, 256
HPC = 3              # heads per core
DHC = HPC * DH       # 192 head-dims per core
NCORES = 8
C2 = 256             # query chunk
NCH = S // C2        # 16 chunks
NKT = S // 128       # 32 key tiles
SBLK = 512           # projection s-block
NSB = S // SBLK      # 8 s-blocks
VAW = DH + 1         # 65: V columns + ones column
MQK = 2 * DHC        # 384 fused q+k output dims
NSL = 5              # score slots per chunk (slot 0 = folded edges)
SCW = NSL * C2       # 1280 score columns per (chunk, head)
AluOp = mybir.AluOpType
ActFn = mybir.ActivationFunctionType
NPBF16 = ml_dtypes.bfloat16


def _build_program(use_b, use_bv, use_fmask, use_qmask):
    nc = bacc.Bacc("TRN2", num_devices=NCORES)

    xb_d = nc.dram_tensor("xb", (S, D), BF16, kind="ExternalInput").ap()
    wqk_d = nc.dram_tensor("wqk", (D, MQK), BF16, kind="ExternalInput").ap()
    wv_d = nc.dram_tensor("wv", (D, DHC), BF16, kind="ExternalInput").ap()
    tge_d = nc.dram_tensor("t_ge", (128, 128), BF16, kind="ExternalInput").ap()
    tgl_d = nc.dram_tensor("t_gl", (128, 256), BF16, kind="ExternalInput").ap()
    tle_d = nc.dram_tensor("t_le", (128, 128), BF16, kind="ExternalInput").ap()
    if use_b:
        bqk_d = nc.dram_tensor("bqk", (MQK, 1), F32, kind="ExternalInput").ap()
    if use_bv:
        bvr_d = nc.dram_tensor("bvr", (1, DHC), BF16, kind="ExternalInput").ap()
    if use_fmask:
        fmk_d = nc.dram_tensor("fmk", (128, NKT), F32, kind="ExternalInput").ap()
    if use_qmask:
        qmk_d = nc.dram_tensor("qmk", (128, NKT), F32, kind="ExternalInput").ap()
    out_d = nc.dram_tensor("out", (S, DHC), F32, kind="ExternalOutput").ap()

    with tile.TileContext(nc) as tc, ExitStack() as ctx:
        pers = ctx.enter_context(tc.tile_pool(name="pers", bufs=1))

        wqk = pers.tile([128, 6 * MQK], BF16, tag="wqk", name="wqk")
        nc.sync.dma_start(wqk[:], wqk_d.rearrange("(a p) n -> p a n", p=128))
        wqk3 = wqk.rearrange("p (a n) -> p a n", a=6)
        wv = pers.tile([128, 6 * DHC], BF16, tag="wv", name="wv")
        nc.sync.dma_start(wv[:], wv_d.rearrange("(a p) n -> p a n", p=128))
        wv3 = wv.rearrange("p (a n) -> p a n", a=6)
        t_ge = pers.tile([128, 128], BF16, tag="t_ge", name="t_ge")
        t_gl = pers.tile([128, 256], BF16, tag="t_gl", name="t_gl")
        t_le = pers.tile([128, 128], BF16, tag="t_le", name="t_le")
        nc.sync.dma_start(t_ge[:], tge_d)
        nc.sync.dma_start(t_gl[:], tgl_d)
        nc.sync.dma_start(t_le[:], tle_d)
        if use_b:
            bqk = pers.tile([128, 3], F32, tag="bqk", name="bqk")
            nc.sync.dma_start(bqk[:], bqk_d.rearrange("(m p) c -> p (m c)", p=128))
        if use_bv:
            bvr = pers.tile([1, DHC], BF16, tag="bvr", name="bvr")
            nc.sync.dma_start(bvr[:], bvr_d)
        if use_fmask:
            fmk = pers.tile([128, NKT], F32, tag="fmk", name="fmk")
            nc.sync.dma_start(fmk[:], fmk_d)
        if use_qmask:
            qmk = pers.tile([128, NKT], F32, tag="qmk", name="qmk")
            nc.sync.dma_start(qmk[:], qmk_d)

        # persistent activations
        xT = pers.tile([128, 6 * S], BF16, tag="xT", name="xT")
        xT3 = xT.rearrange("p (a s) -> p a s", a=6)
        qT01 = pers.tile([128, S], BF16, tag="qT01", name="qT01")
        kT01 = pers.tile([128, S], BF16, tag="kT01", name="kT01")
        qT2 = pers.tile([64, S], BF16, tag="qT2", name="qT2")
        kT2 = pers.tile([64, S], BF16, tag="kT2", name="kT2")
        va = pers.tile([128, NKT * HPC * VAW], BF16, tag="va", name="va")
        va4 = va.rearrange("p (t h c) -> p t h c", h=HPC, c=VAW)
        nc.gpsimd.memset(va4[:, :, :, DH:VAW], 1.0)
        # pre-warm the exp table while phase 1 runs
        scr = pers.tile([1, 1], BF16, tag="scr", name="scr")
        nc.scalar.activation(scr[:], t_ge[0:1, 0:1], ActFn.Exp)

        # ---------------- phase 1: X^T, projections, V_aug ----------------
        with tc.tile_pool(name="psB", bufs=2, space="PSUM") as psB, \
             tc.tile_pool(name="psC", bufs=4, space="PSUM") as psC:
            for sb in range(NSB):
                ssl = slice(sb * SBLK, (sb + 1) * SBLK)
                for dt in range(6):
                    nc.sync.dma_start_transpose(
                        xT3[:, dt, ssl],
                        xb_d[ssl, dt * 128:(dt + 1) * 128])
                # fused Q/K projection: m0 = q dims 0..127 (h0,h1),
                # m1 = k dims 0..127 (h0,h1), m2 = q h2 | k h2
                for m in range(3):
                    ps = psB.tile([128, SBLK], F32, tag="pb", name=f"pb{m}_{sb}",
                                  bufs=2)
                    for k in range(6):
                        nc.tensor.matmul(
                            ps[:],
                            wqk3[:, k, m * 128:(m + 1) * 128],
                            xT3[:, k, ssl],
                            start=(k == 0), stop=(k == 5))
                    if m == 0:
                        dsts = [(qT01[:, ssl], ps[:], None)]
                    elif m == 1:
                        dsts = [(kT01[:, ssl], ps[:], None)]
                    else:
                        dsts = [(qT2[:, ssl], ps[0:64, :], (0, 64)),
                                (kT2[:, ssl], ps[64:128, :], (64, 128))]
                    for dst, src, rows in dsts:
                        if use_b:
                            if rows is None:
                                bias = bqk[:, m:m + 1]
                            else:
                                bias = bqk[rows[0]:rows[1], m:m + 1]
                            nc.vector.tensor_scalar_add(dst, src, bias)
                        else:
                            nc.vector.tensor_copy(dst, src)
                # V row-major into V_aug
                for st in range(sb * 4, sb * 4 + 4):
                    vps = psC.tile([128, DHC], F32, tag="vps", name="vps",
                                   bufs=4)
                    for k in range(6):
                        nc.tensor.matmul(
                            vps[:],
                            xT3[:, k, st * 128:(st + 1) * 128],
                            wv3[:, k, :],
                            start=(k == 0), stop=(k == 5 and not use_bv))
                    if use_bv:
                        # bias via K=1 ones-row matmul (t_ge row 127 = ones)
                        nc.tensor.matmul(
                            vps[:], t_ge[127:128, 0:128], bvr[0:1, :],
                            start=False, stop=True)
                    nc.vector.tensor_copy(
                        va4[:, st, :, 0:DH],
                        vps.rearrange("p (h d) -> p h d", h=HPC))

        # ---------------- phase 2: banded attention ----------------
        # slot layout per (chunk, head): slot 0 cols [0:128] = edge tile
        # j=-2 half0 (t_ge), cols [128:256] = edge tile j=3 half1 (t_le);
        # slots 1..4 = full tiles j=-1..2.
        def slots_for(ci):
            """[(slot, col0, ncol, kt, half_or_None)]"""
            out = []
            kt_lo = 2 * ci - 2
            if kt_lo >= 0:
                out.append((0, 0, 128, kt_lo, 0))
            kt_hi = 2 * ci + 3
            if kt_hi <= NKT - 1:
                out.append((0, 128, 128, kt_hi, 1))
            for j in range(-1, 3):
                kt = 2 * ci + j
                if 0 <= kt <= NKT - 1:
                    out.append((1 + j + 1, (j + 2) * C2, C2, kt, None))
            return out

        with tc.tile_pool(name="pD", bufs=1) as pD, \
             tc.tile_pool(name="psD_sc", bufs=2, space="PSUM") as psD_sc, \
             tc.tile_pool(name="psD_av", bufs=2, space="PSUM") as psD_av:
            state = {}

            def front(ci):
                qsl = slice(ci * C2, (ci + 1) * C2)
                sl = slots_for(ci)
                scs = []
                pts = []
                for h in range(HPC):
                    scs.append(psD_sc.tile([128, SCW], F32, tag="sc",
                                           name=f"sc{ci}_{h}", bufs=2))
                for _, c0, nc_, kt, hf in sl:
                    ksl = slice(kt * 128, (kt + 1) * 128)
                    if hf is None:
                        q0 = qsl
                    else:
                        q0 = slice(ci * C2 + hf * 128, ci * C2 + hf * 128 + 128)
                    osl = slice(c0, c0 + nc_)
                    nc.tensor.matmul(scs[0][:, osl], kT01[0:64, ksl],
                                     qT01[0:64, q0], start=True, stop=True)
                    nc.tensor.matmul(scs[1][:, osl], kT01[64:128, ksl],
                                     qT01[64:128, q0], start=True, stop=True)
                    nc.tensor.matmul(scs[2][:, osl], kT2[0:64, ksl],
                                     qT2[0:64, q0], start=True, stop=True)
                lo = 0 if (2 * ci - 2 >= 0 or 2 * ci + 3 <= NKT - 1) else C2
                hi = SCW if 2 * ci + 2 <= NKT - 1 else 4 * C2
                for h in range(HPC):
                    pt = pD.tile([128, SCW], BF16, tag="pt",
                                 name=f"pt{ci}_{h}", bufs=6)
                    nc.scalar.activation(pt[:, lo:hi], scs[h][:, lo:hi],
                                         ActFn.Exp)
                    pts.append(pt)
                state[ci] = pts

            def mid(ci):
                pts = state[ci]
                sl = slots_for(ci)
                has_lo = 2 * ci - 2 >= 0
                has_hi = 2 * ci + 3 <= NKT - 1
                av_big = psD_av.tile([128, 6 * VAW], F32, tag="av",
                                     name=f"av{ci}", bufs=2)
                av6 = av_big.rearrange("p (g c) -> p g c", c=VAW)
                state[(ci, 'av')] = av6
                for h in range(HPC):
                    pt = pts[h]
                    if has_lo and has_hi:
                        nc.vector.tensor_tensor(pt[:, 0:256], pt[:, 0:256],
                                                t_gl[:], op=AluOp.mult)
                    elif has_lo:
                        nc.vector.tensor_tensor(pt[:, 0:128], pt[:, 0:128],
                                                t_ge[:], op=AluOp.mult)
                    elif has_hi:
                        nc.vector.tensor_tensor(pt[:, 128:256], pt[:, 128:256],
                                                t_le[:], op=AluOp.mult)
                    # full-tile edge masks: j=-1 half1 (t_ge), j=2 half0 (t_le)
                    for slot, c0, nc_, kt, hf in sl:
                        j = kt - 2 * ci
                        if hf is not None:
                            continue
                        if j == -1:
                            nc.vector.tensor_tensor(
                                pt[:, c0 + 128:c0 + 256],
                                pt[:, c0 + 128:c0 + 256], t_ge[:],
                                op=AluOp.mult)
                        elif j == 2:
                            nc.vector.tensor_tensor(
                                pt[:, c0:c0 + 128], pt[:, c0:c0 + 128],
                                t_le[:], op=AluOp.mult)
                    if use_fmask:
                        for slot, c0, nc_, kt, hf in sl:
                            nc.vector.tensor_scalar_mul(
                                pt[:, c0:c0 + nc_], pt[:, c0:c0 + nc_],
                                fmk[:, kt:kt + 1])
                    # AV accumulation
                    for hf in range(2):
                        lst = []
                        for slot, c0, nc_, kt, shf in sl:
                            if shf is None:
                                lst.append((kt, pt[:, c0 + hf * 128:
                                                   c0 + hf * 128 + 128]))
                            elif shf == hf:
                                lst.append((kt, pt[:, c0:c0 + 128]))
                        lst.sort()
                        g = h * 2 + hf
                        for i, (kt, psl) in enumerate(lst):
                            nc.tensor.matmul(
                                av6[:, g, :], psl, va4[:, kt, h, :],
                                start=(i == 0), stop=(i == len(lst) - 1))

            def tail(ci):
                av6 = state.pop((ci, 'av'))
                state.pop(ci)
                os_t = [pD.tile([128, DHC], F32, tag="os", name=f"os{ci}_{hf}",
                                bufs=4) for hf in range(2)]
                rzs = pD.tile([128, 6], F32, tag="rzs", name=f"rzs{ci}",
                              bufs=3)
                nc.vector.reciprocal(rzs[:], av6[:, :, DH])
                if use_qmask:
                    for g in range(6):
                        nc.vector.tensor_scalar_mul(
                            rzs[:, g:g + 1], rzs[:, g:g + 1],
                            qmk[:, 2 * ci + (g % 2):2 * ci + (g % 2) + 1])
                for h in range(HPC):
                    for hf in range(2):
                        g = h * 2 + hf
                        nc.vector.tensor_scalar_mul(
                            os_t[hf][:, h * DH:(h + 1) * DH],
                            av6[:, g, 0:DH], rzs[:, g:g + 1])
                for hf in range(2):
                    qt = 2 * ci + hf
                    nc.sync.dma_start(
                        out_d[qt * 128:(qt + 1) * 128, :], os_t[hf][:])

            for ci in range(NCH):
                front(ci)
                if ci >= 1:
                    mid(ci - 1)
                if ci >= 2:
                    tail(ci - 2)
            mid(NCH - 1)
            tail(NCH - 2)
            tail(NCH - 1)

    nc.compile()
    return nc


_prog_cache = {}


def _get_program(use_b, use_bv, use_fmask, use_qmask):
    key = (use_b, use_bv, use_fmask, use_qmask)
    if key not in _prog_cache:
        _prog_cache[key] = _build_program(use_b, use_bv, use_fmask, use_qmask)
    return _prog_cache[key]


def _host_constants():
    kl = np.arange(128)[:, None]
    ql = np.arange(128)[None, :]
    t_ge = (kl >= ql).astype(NPBF16)
    t_le = (kl <= ql).astype(NPBF16)
    t_gl = np.concatenate([t_ge, t_le], axis=1)
    return t_ge, t_gl, t_le


def kernel(hidden_states, attention_mask, is_index_masked, Wq, bq, Wk, bk, Wv, bv,
           trace=False):
    hidden_states = np.asarray(hidden_states, dtype=np.float32)
    attention_mask = np.asarray(attention_mask, dtype=np.float32)
    is_index_masked = np.asarray(is_index_masked)
    Wq = np.asarray(Wq, dtype=np.float32)
    Wk = np.asarray(Wk, dtype=np.float32)
    Wv = np.asarray(Wv, dtype=np.float32)
    bq = np.asarray(bq, dtype=np.float32)
    bk = np.asarray(bk, dtype=np.float32)
    bv = np.asarray(bv, dtype=np.float32)

    use_b = bool(np.any(bq != 0) or np.any(bk != 0))
    use_bv = bool(np.any(bv != 0))
    use_fmask = bool(np.any(attention_mask != 0))
    use_qmask = bool(np.any(is_index_masked))
    nc = _get_program(use_b, use_bv, use_fmask, use_qmask)

    scale = 1.0 / math.sqrt(DH)
    t_ge, t_gl, t_le = _host_constants()

    xb = hidden_states.astype(NPBF16)

    in_maps = []
    for cid in range(NCORES):
        b = cid // 4
        h0 = HPC * (cid % 4)
        c0, c1 = h0 * DH, (h0 + HPC) * DH
        wql = Wq[:, c0:c1] * scale
        wkl = Wk[:, c0:c1]
        wqk_h = np.concatenate(
            [wql[:, 0:128], wkl[:, 0:128], wql[:, 128:192], wkl[:, 128:192]],
            axis=1).astype(NPBF16)
        m = {
            "xb": xb[b],
            "wqk": np.ascontiguousarray(wqk_h),
            "wv": np.ascontiguousarray(Wv[:, c0:c1].astype(NPBF16)),
            "t_ge": t_ge,
            "t_gl": t_gl,
            "t_le": t_le,
        }
        if use_b:
            bql = bq[c0:c1] * scale
            bkl = bk[c0:c1]
            m["bqk"] = np.ascontiguousarray(np.concatenate(
                [bql[0:128], bkl[0:128], bql[128:192], bkl[128:192]])
                .reshape(MQK, 1))
        if use_bv:
            m["bvr"] = np.ascontiguousarray(
                bv[c0:c1].astype(NPBF16).reshape(1, DHC))
        if use_fmask:
            fac = (attention_mask[b] == 0).astype(np.float32)
            m["fmk"] = np.ascontiguousarray(fac.reshape(NKT, 128).T)
        if use_qmask:
            keep = (~is_index_masked[b]).astype(np.float32)
            m["qmk"] = np.ascontiguousarray(keep.reshape(NKT, 128).T)
        in_maps.append(m)

    res = run_bass_kernel_spmd(nc, in_maps, core_ids=list(range(NCORES)),
                               trace=trace)
    out = np.empty((B, S, D), dtype=np.float32)
    for cid in range(NCORES):
        b = cid // 4
        h0 = HPC * (cid % 4)
        out[b, :, h0 * DH:(h0 + HPC) * DH] = res.results[cid]["out"]
    if trace:
        return out, res
    return out


# revision 22
# speedup vs baseline: 1.8928x; 1.1425x over previous
"""Longformer sliding-window self-attention (B=2, S=4096, D=768, H=12, Dh=64,
one-sided window W=256) on 8 TRN2 NeuronCores.

Sharding: (batch, head-group) — core = b*4 + g handles batch b, heads
[3g, 3g+3). All-bf16 operand path (f32 PSUM accumulation):

  phase 1 (per 512-row s-block, pipelined):
    X^T loaded directly via DMA xbar transpose (host pre-casts X to bf16),
    fused Q/K projection W_qk^T @ X^T (bf16, full 128-row m-tiles),
    V computed row-major (X^T tiles stationary, Wv moving) into V_aug
    [s, kt, h, 65] with a ones column (softmax denominator). Q^T/K^T land
    in per-head zero-padded [128, S] tiles (head data rows 0-63, zeros
    64-127) so the score matmuls run with full K=128 stationary operands
    (fast weight load + LDWEIGHTS/matmul overlap). Row halves that the
    projection emits on PSUM partitions 64-127 are staged and moved down
    by SBUF->SBUF DMA (cross-partition moves need the DMA engines).
  phase 2 (per 256-query chunk, lag-2 software pipeline):
    folded-edge banded scores S^T[k, q]: 4 full key tiles + the two edge
    half-tiles packed into one 256-col slot (slot 0), exp on ACT straight
    out of PSUM, band-edge masking via t_ge/t_le multiplies on DVE (bf16
    2x mode), AV accumulation with ones-column Z, output rows scaled by
    1/Z on DVE. Emission order: scores/exp(ci), masks/AV(ci-1),
    recip/epilogue/store(ci-2) — keeps every engine queue stall-free.

kernel() takes full inputs, shards, runs SPMD on cores 0..7, reassembles.
"""
import sys

if '/opt/trn_rl_repo' not in sys.path:
    sys.path.insert(0, '/opt/trn_rl_repo')

import math
from contextlib import ExitStack

import numpy as np
import ml_dtypes

import concourse.bacc as bacc
import concourse.mybir as mybir
import concourse.tile as tile
from concourse.bass_utils import run_bass_kernel_spmd

F32 = mybir.dt.float32
BF16 = mybir.dt.bfloat16

B, S, D = 2, 4096, 768
H, DH, W = 12, 64, 256
HPC = 3              # heads per core
DHC = HPC * DH       # 192 head-dims per core
NCORES = 8
C2 = 256             # query chunk
NCH = S // C2        # 16 chunks
NKT = S // 128       # 32 key tiles
SBLK = 512           # projection s-block
NSB = S // SBLK      # 8 s-blocks
VAW = DH + 1         # 65: V columns + ones column
MQK = 2 * DHC        # 384 fused q+k output dims
NSL = 5              # score slots per chunk (slot 0 = folded edges)
SCW = NSL * C2       # 1280 score columns per (chunk, head)
AluOp = mybir.AluOpType
ActFn = mybir.ActivationFunctionType
NPBF16 = ml_dtypes.bfloat16


def _build_program(use_b, use_bv, use_fmask, use_qmask):
    nc = bacc.Bacc("TRN2", num_devices=NCORES)

    xb_d = nc.dram_tensor("xb", (S, D), BF16, kind="ExternalInput").ap()
    wqk_d = nc.dram_tensor("wqk", (D, MQK), BF16, kind="ExternalInput").ap()
    wv_d = nc.dram_tensor("wv", (D, DHC), BF16, kind="ExternalInput").ap()
    tge_d = nc.dram_tensor("t_ge", (128, 128), BF16, kind="ExternalInput").ap()
    tgl_d = nc.dram_tensor("t_gl", (128, 256), BF16, kind="ExternalInput").ap()
    tle_d = nc.dram_tensor("t_le", (128, 128), BF16, kind="ExternalInput").ap()
    if use_b:
        bqk_d = nc.dram_tensor("bqk", (MQK, 1), F32, kind="ExternalInput").ap()
    if use_bv:
        bvr_d = nc.dram_tensor("bvr", (1, DHC), BF16, kind="ExternalInput").ap()
    if use_fmask:
        fmk_d = nc.dram_tensor("fmk", (128, NKT), F32, kind="ExternalInput").ap()
    if use_qmask:
        qmk_d = nc.dram_tensor("qmk", (128, NKT), F32, kind="ExternalInput").ap()
    out_d = nc.dram_tensor("out", (S, DHC), F32, kind="ExternalOutput").ap()

    with tile.TileContext(nc) as tc, ExitStack() as ctx:
        pers = ctx.enter_context(tc.tile_pool(name="pers", bufs=1))

        wqk = pers.tile([128, 6 * MQK], BF16, tag="wqk", name="wqk")
        nc.sync.dma_start(wqk[:], wqk_d.rearrange("(a p) n -> p a n", p=128))
        wqk3 = wqk.rearrange("p (a n) -> p a n", a=6)
        wv = pers.tile([128, 6 * DHC], BF16, tag="wv", name="wv")
        nc.sync.dma_start(wv[:], wv_d.rearrange("(a p) n -> p a n", p=128))
        wv3 = wv.rearrange("p (a n) -> p a n", a=6)
        t_ge = pers.tile([128, 128], BF16, tag="t_ge", name="t_ge")
        t_gl = pers.tile([128, 256], BF16, tag="t_gl", name="t_gl")
        t_le = pers.tile([128, 128], BF16, tag="t_le", name="t_le")
        nc.sync.dma_start(t_ge[:], tge_d)
        nc.sync.dma_start(t_gl[:], tgl_d)
        nc.sync.dma_start(t_le[:], tle_d)
        if use_b:
            bqk = pers.tile([128, 3], F32, tag="bqk", name="bqk")
            nc.sync.dma_start(bqk[:], bqk_d.rearrange("(m p) c -> p (m c)", p=128))
        if use_bv:
            bvr = pers.tile([1, DHC], BF16, tag="bvr", name="bvr")
            nc.sync.dma_start(bvr[:], bvr_d)
        if use_fmask:
            fmk = pers.tile([128, NKT], F32, tag="fmk", name="fmk")
            nc.sync.dma_start(fmk[:], fmk_d)
        if use_qmask:
            qmk = pers.tile([128, NKT], F32, tag="qmk", name="qmk")
            nc.sync.dma_start(qmk[:], qmk_d)

        # persistent activations
        xT = pers.tile([128, 6 * S], BF16, tag="xT", name="xT")
        xT3 = xT.rearrange("p (a s) -> p a s", a=6)
        # per-head zero-padded Q^T/K^T: head data rows 0-63, zeros 64-127
        qz = [pers.tile([128, S], BF16, tag=f"qz{h}", name=f"qz{h}")
              for h in range(HPC)]
        kz = [pers.tile([128, S], BF16, tag=f"kz{h}", name=f"kz{h}")
              for h in range(HPC)]
        # staging for the projection outputs that land on PSUM rows 64-127
        # (q-h1, k-h1, k-h2); moved to rows 0-63 of qz/kz by SBUF->SBUF DMA
        stg = [pers.tile([128, S], BF16, tag=f"stg{i}", name=f"stg{i}")
               for i in range(3)]
        for t in qz + kz:
            nc.gpsimd.memset(t[64:128, :], 0.0)
        va = pers.tile([128, NKT * HPC * VAW], BF16, tag="va", name="va")
        va4 = va.rearrange("p (t h c) -> p t h c", h=HPC, c=VAW)
        nc.gpsimd.memset(va4[:, :, :, DH:VAW], 1.0)
        # pre-warm the exp table while phase 1 runs
        scr = pers.tile([1, 1], BF16, tag="scr", name="scr")
        nc.scalar.activation(scr[:], t_ge[0:1, 0:1], ActFn.Exp)

        # ---------------- phase 1: X^T, projections, V_aug ----------------
        with tc.tile_pool(name="psB", bufs=2, space="PSUM") as psB, \
             tc.tile_pool(name="psC", bufs=4, space="PSUM") as psC:
            for sb in range(NSB):
                ssl = slice(sb * SBLK, (sb + 1) * SBLK)
                for dt in range(6):
                    nc.sync.dma_start_transpose(
                        xT3[:, dt, ssl],
                        xb_d[ssl, dt * 128:(dt + 1) * 128])
                # fused Q/K projection: m0 = q h0 | q h1, m1 = k h0 | k h1,
                # m2 = q h2 | k h2
                for m in range(3):
                    ps = psB.tile([128, SBLK], F32, tag="pb", name=f"pb{m}_{sb}",
                                  bufs=2)
                    for k in range(6):
                        nc.tensor.matmul(
                            ps[:],
                            wqk3[:, k, m * 128:(m + 1) * 128],
                            xT3[:, k, ssl],
                            start=(k == 0), stop=(k == 5))
                    if m == 0:
                        dsts = [(qz[0][0:64, ssl], ps[0:64, :], (0, 64)),
                                (stg[0][64:128, ssl], ps[64:128, :], (64, 128))]
                    elif m == 1:
                        dsts = [(kz[0][0:64, ssl], ps[0:64, :], (0, 64)),
                                (stg[1][64:128, ssl], ps[64:128, :], (64, 128))]
                    else:
                        dsts = [(qz[2][0:64, ssl], ps[0:64, :], (0, 64)),
                                (stg[2][64:128, ssl], ps[64:128, :], (64, 128))]
                    for dst, src, rows in dsts:
                        if use_b:
                            bias = bqk[rows[0]:rows[1], m:m + 1]
                            nc.vector.tensor_scalar_add(dst, src, bias)
                        else:
                            nc.vector.tensor_copy(dst, src)
                # V row-major into V_aug
                for st in range(sb * 4, sb * 4 + 4):
                    vps = psC.tile([128, DHC], F32, tag="vps", name="vps",
                                   bufs=4)
                    for k in range(6):
                        nc.tensor.matmul(
                            vps[:],
                            xT3[:, k, st * 128:(st + 1) * 128],
                            wv3[:, k, :],
                            start=(k == 0), stop=(k == 5 and not use_bv))
                    if use_bv:
                        # bias via K=1 ones-row matmul (t_ge row 127 = ones)
                        nc.tensor.matmul(
                            vps[:], t_ge[127:128, 0:128], bvr[0:1, :],
                            start=False, stop=True)
                    nc.vector.tensor_copy(
                        va4[:, st, :, 0:DH],
                        vps.rearrange("p (h d) -> p h d", h=HPC))
            # move staged halves down to rows 0-63 of their padded tiles
            nc.sync.dma_start(qz[1][0:64, :], stg[0][64:128, :])
            nc.sync.dma_start(kz[1][0:64, :], stg[1][64:128, :])
            nc.sync.dma_start(kz[2][0:64, :], stg[2][64:128, :])

        # ---------------- phase 2: banded attention ----------------
        # slot layout per (chunk, head): slot 0 cols [0:128] = edge tile
        # j=-2 half0 (t_ge), cols [128:256] = edge tile j=3 half1 (t_le);
        # slots 1..4 = full tiles j=-1..2.
        def slots_for(ci):
            """[(slot, col0, ncol, kt, half_or_None)]"""
            out = []
            kt_lo = 2 * ci - 2
            if kt_lo >= 0:
                out.append((0, 0, 128, kt_lo, 0))
            kt_hi = 2 * ci + 3
            if kt_hi <= NKT - 1:
                out.append((0, 128, 128, kt_hi, 1))
            for j in range(-1, 3):
                kt = 2 * ci + j
                if 0 <= kt <= NKT - 1:
                    out.append((1 + j + 1, (j + 2) * C2, C2, kt, None))
            return out

        with tc.tile_pool(name="pD", bufs=1) as pD, \
             tc.tile_pool(name="psD_sc", bufs=2, space="PSUM") as psD_sc, \
             tc.tile_pool(name="psD_av", bufs=2, space="PSUM") as psD_av:
            state = {}

            def front(ci):
                qsl = slice(ci * C2, (ci + 1) * C2)
                sl = slots_for(ci)
                scs = [psD_sc.tile([128, SCW], F32, tag="sc",
                                   name=f"sc{ci}_{h}", bufs=2)
                       for h in range(HPC)]
                for _, c0, nc_, kt, hf in sl:
                    ksl = slice(kt * 128, (kt + 1) * 128)
                    if hf is None:
                        q0 = qsl
                    else:
                        q0 = slice(ci * C2 + hf * 128, ci * C2 + hf * 128 + 128)
                    osl = slice(c0, c0 + nc_)
                    for h in range(HPC):
                        nc.tensor.matmul(scs[h][:, osl], kz[h][:, ksl],
                                         qz[h][:, q0], start=True, stop=True)
                hi = SCW if 2 * ci + 2 <= NKT - 1 else 4 * C2
                pts = []
                for h in range(HPC):
                    pt = pD.tile([128, SCW], BF16, tag="pt",
                                 name=f"pt{ci}_{h}", bufs=6)
                    nc.scalar.activation(pt[:, 0:hi], scs[h][:, 0:hi],
                                         ActFn.Exp)
                    pts.append(pt)
                state[ci] = pts

            def mid(ci):
                pts = state[ci]
                sl = slots_for(ci)
                has_lo = 2 * ci - 2 >= 0
                has_hi = 2 * ci + 3 <= NKT - 1
                av_big = psD_av.tile([128, 6 * VAW], F32, tag="av",
                                     name=f"av{ci}", bufs=2)
                av6 = av_big.rearrange("p (g c) -> p g c", c=VAW)
                state[(ci, 'av')] = av6
                for h in range(HPC):
                    pt = pts[h]
                    if has_lo and has_hi:
                        nc.vector.tensor_tensor(pt[:, 0:256], pt[:, 0:256],
                                                t_gl[:], op=AluOp.mult)
                    elif has_lo:
                        nc.vector.tensor_tensor(pt[:, 0:128], pt[:, 0:128],
                                                t_ge[:], op=AluOp.mult)
                    elif has_hi:
                        nc.vector.tensor_tensor(pt[:, 128:256], pt[:, 128:256],
                                                t_le[:], op=AluOp.mult)
                    # full-tile edge masks: j=-1 half1 (t_ge), j=2 half0 (t_le)
                    for slot, c0, nc_, kt, hf in sl:
                        j = kt - 2 * ci
                        if hf is not None:
                            continue
                        if j == -1:
                            nc.vector.tensor_tensor(
                                pt[:, c0 + 128:c0 + 256],
                                pt[:, c0 + 128:c0 + 256], t_ge[:],
                                op=AluOp.mult)
                        elif j == 2:
                            nc.vector.tensor_tensor(
                                pt[:, c0:c0 + 128], pt[:, c0:c0 + 128],
                                t_le[:], op=AluOp.mult)
                    if use_fmask:
                        for slot, c0, nc_, kt, hf in sl:
                            nc.vector.tensor_scalar_mul(
                                pt[:, c0:c0 + nc_], pt[:, c0:c0 + nc_],
                                fmk[:, kt:kt + 1])
                    # AV accumulation
                    for hf in range(2):
                        lst = []
                        for slot, c0, nc_, kt, shf in sl:
                            if shf is None:
                                lst.append((kt, pt[:, c0 + hf * 128:
                                                   c0 + hf * 128 + 128]))
                            elif shf == hf:
                                lst.append((kt, pt[:, c0:c0 + 128]))
                        lst.sort(key=lambda x: x[0])
                        g = h * 2 + hf
                        for i, (kt, psl) in enumerate(lst):
                            nc.tensor.matmul(
                                av6[:, g, :], psl, va4[:, kt, h, :],
                                start=(i == 0), stop=(i == len(lst) - 1))

            def tail(ci):
                av6 = state.pop((ci, 'av'))
                state.pop(ci)
                os_t = [pD.tile([128, DHC], F32, tag="os", name=f"os{ci}_{hf}",
                                bufs=4) for hf in range(2)]
                rzs = pD.tile([128, 6], F32, tag="rzs", name=f"rzs{ci}",
                              bufs=3)
                nc.vector.reciprocal(rzs[:], av6[:, :, DH])
                if use_qmask:
                    for g in range(6):
                        nc.vector.tensor_scalar_mul(
                            rzs[:, g:g + 1], rzs[:, g:g + 1],
                            qmk[:, 2 * ci + (g % 2):2 * ci + (g % 2) + 1])
                for h in range(HPC):
                    for hf in range(2):
                        g = h * 2 + hf
                        nc.vector.tensor_scalar_mul(
                            os_t[hf][:, h * DH:(h + 1) * DH],
                            av6[:, g, 0:DH], rzs[:, g:g + 1])
                for hf in range(2):
                    qt = 2 * ci + hf
                    nc.sync.dma_start(
                        out_d[qt * 128:(qt + 1) * 128, :], os_t[hf][:])

            for ci in range(NCH):
                front(ci)
                if ci >= 1:
                    mid(ci - 1)
                if ci >= 2:
                    tail(ci - 2)
            mid(NCH - 1)
            tail(NCH - 2)
            tail(NCH - 1)

    nc.compile()
    return nc


_prog_cache = {}


def _get_program(use_b, use_bv, use_fmask, use_qmask):
    key = (use_b, use_bv, use_fmask, use_qmask)
    if key not in _prog_cache:
        _prog_cache[key] = _build_program(use_b, use_bv, use_fmask, use_qmask)
    return _prog_cache[key]


def _host_constants():
    kl = np.arange(128)[:, None]
    ql = np.arange(128)[None, :]
    t_ge = (kl >= ql).astype(NPBF16)
    t_le = (kl <= ql).astype(NPBF16)
    t_gl = np.concatenate([t_ge, t_le], axis=1)
    return t_ge, t_gl, t_le


def kernel(hidden_states, attention_mask, is_index_masked, Wq, bq, Wk, bk, Wv, bv,
           trace=False):
    hidden_states = np.asarray(hidden_states, dtype=np.float32)
    attention_mask = np.asarray(attention_mask, dtype=np.float32)
    is_index_masked = np.asarray(is_index_masked)
    Wq = np.asarray(Wq, dtype=np.float32)
    Wk = np.asarray(Wk, dtype=np.float32)
    Wv = np.asarray(Wv, dtype=np.float32)
    bq = np.asarray(bq, dtype=np.float32)
    bk = np.asarray(bk, dtype=np.float32)
    bv = np.asarray(bv, dtype=np.float32)

    use_b = bool(np.any(bq != 0) or np.any(bk != 0))
    use_bv = bool(np.any(bv != 0))
    use_fmask = bool(np.any(attention_mask != 0))
    use_qmask = bool(np.any(is_index_masked))
    nc = _get_program(use_b, use_bv, use_fmask, use_qmask)

    scale = 1.0 / math.sqrt(DH)
    t_ge, t_gl, t_le = _host_constants()

    xb = hidden_states.astype(NPBF16)

    in_maps = []
    for cid in range(NCORES):
        b = cid // 4
        h0 = HPC * (cid % 4)
        c0, c1 = h0 * DH, (h0 + HPC) * DH
        wql = Wq[:, c0:c1] * scale
        wkl = Wk[:, c0:c1]
        wqk_h = np.concatenate(
            [wql[:, 0:128], wkl[:, 0:128], wql[:, 128:192], wkl[:, 128:192]],
            axis=1).astype(NPBF16)
        m = {
            "xb": xb[b],
            "wqk": np.ascontiguousarray(wqk_h),
            "wv": np.ascontiguousarray(Wv[:, c0:c1].astype(NPBF16)),
            "t_ge": t_ge,
            "t_gl": t_gl,
            "t_le": t_le,
        }
        if use_b:
            bql = bq[c0:c1] * scale
            bkl = bk[c0:c1]
            m["bqk"] = np.ascontiguousarray(np.concatenate(
                [bql[0:128], bkl[0:128], bql[128:192], bkl[128:192]])
                .reshape(MQK, 1))
        if use_bv:
            m["bvr"] = np.ascontiguousarray(
                bv[c0:c1].astype(NPBF16).reshape(1, DHC))
        if use_fmask:
            fac = (attention_mask[b] == 0).astype(np.float32)
            m["fmk"] = np.ascontiguousarray(fac.reshape(NKT, 128).T)
        if use_qmask:
            keep = (~is_index_masked[b]).astype(np.float32)
            m["qmk"] = np.ascontiguousarray(keep.reshape(NKT, 128).T)
        in_maps.append(m)

    res = run_bass_kernel_spmd(nc, in_maps, core_ids=list(range(NCORES)),
                               trace=trace)
    out = np.empty((B, S, D), dtype=np.float32)
    for cid in range(NCORES):
        b = cid // 4
        h0 = HPC * (cid % 4)
        out[b, :, h0 * DH:(h0 + HPC) * DH] = res.results[cid]["out"]
    if trace:
        return out, res
    return out


# revision 23
# speedup vs baseline: 2.0388x; 1.0772x over previous
"""Longformer sliding-window self-attention (B=2, S=4096, D=768, H=12, Dh=64,
one-sided window W=256) on 8 TRN2 NeuronCores.

Sharding: (batch, head-group) — core = b*4 + g handles batch b, heads
[3g, 3g+3). All-bf16 operand path (f32 PSUM accumulation):

  phase 1 (per 512-row s-block, pipelined):
    X^T loaded directly via DMA xbar transpose (host pre-casts X to bf16),
    fused Q/K projection W_qk^T @ X^T (bf16, full 128-row m-tiles),
    V computed row-major (X^T tiles stationary, Wv moving) into V_aug
    [s, kt, h, 65] with a ones column (softmax denominator). Q^T/K^T land
    in per-head zero-padded [128, S] tiles (head data rows 0-63, zeros
    64-127) so the score matmuls run with full K=128 stationary operands
    (fast weight load + LDWEIGHTS/matmul overlap). Row halves that the
    projection emits on PSUM partitions 64-127 are staged and moved down
    by SBUF->SBUF DMA (cross-partition moves need the DMA engines).
  phase 2 (per 256-query chunk, lag-2 software pipeline):
    folded-edge banded scores S^T[k, q]: 4 full key tiles + the two edge
    half-tiles packed into one 256-col slot (slot 0), exp on ACT straight
    out of PSUM, band-edge masking via t_ge/t_le multiplies on DVE (bf16
    2x mode), AV accumulation with ones-column Z, output rows scaled by
    1/Z on DVE. Emission order: scores/exp(ci), masks/AV(ci-1),
    recip/epilogue/store(ci-2) — keeps every engine queue stall-free.

kernel() takes full inputs, shards, runs SPMD on cores 0..7, reassembles.
"""
import sys

if '/opt/trn_rl_repo' not in sys.path:
    sys.path.insert(0, '/opt/trn_rl_repo')

import math
from contextlib import ExitStack

import numpy as np
import ml_dtypes

import concourse.bacc as bacc
import concourse.mybir as mybir
import concourse.tile as tile
from concourse.bass_utils import run_bass_kernel_spmd

F32 = mybir.dt.float32
BF16 = mybir.dt.bfloat16

B, S, D = 2, 4096, 768
H, DH, W = 12, 64, 256
HPC = 3              # heads per core
DHC = HPC * DH       # 192 head-dims per core
NCORES = 8
C2 = 256             # query chunk
NCH = S // C2        # 16 chunks
NKT = S // 128       # 32 key tiles
SBLK = 512           # projection s-block
NSB = S // SBLK      # 8 s-blocks
VAW = DH + 1         # 65: V columns + ones column
MQK = 2 * DHC        # 384 fused q+k output dims
NSL = 5              # score slots per chunk (slot 0 = folded edges)
SCW = NSL * C2       # 1280 score columns per (chunk, head)
AluOp = mybir.AluOpType
ActFn = mybir.ActivationFunctionType
NPBF16 = ml_dtypes.bfloat16


def _build_program(use_b, use_bv, use_fmask, use_qmask):
    nc = bacc.Bacc("TRN2", num_devices=NCORES)

    xb_d = nc.dram_tensor("xb", (S, D), BF16, kind="ExternalInput").ap()
    wqk_d = nc.dram_tensor("wqk", (D, MQK), BF16, kind="ExternalInput").ap()
    wv_d = nc.dram_tensor("wv", (D, DHC), BF16, kind="ExternalInput").ap()
    tge_d = nc.dram_tensor("t_ge", (128, 128), BF16, kind="ExternalInput").ap()
    tgl_d = nc.dram_tensor("t_gl", (128, 256), BF16, kind="ExternalInput").ap()
    tle_d = nc.dram_tensor("t_le", (128, 128), BF16, kind="ExternalInput").ap()
    if use_b:
        bqk_d = nc.dram_tensor("bqk", (MQK, 1), F32, kind="ExternalInput").ap()
    if use_bv:
        bvr_d = nc.dram_tensor("bvr", (1, DHC), BF16, kind="ExternalInput").ap()
    if use_fmask:
        fmk_d = nc.dram_tensor("fmk", (128, NKT), F32, kind="ExternalInput").ap()
    if use_qmask:
        qmk_d = nc.dram_tensor("qmk", (128, NKT), F32, kind="ExternalInput").ap()
    out_d = nc.dram_tensor("out", (S, DHC), F32, kind="ExternalOutput").ap()

    with tile.TileContext(nc) as tc, ExitStack() as ctx:
        pers = ctx.enter_context(tc.tile_pool(name="pers", bufs=1))

        wqk = pers.tile([128, 6 * MQK], BF16, tag="wqk", name="wqk")
        nc.sync.dma_start(wqk[:], wqk_d.rearrange("(a p) n -> p a n", p=128))
        wqk3 = wqk.rearrange("p (a n) -> p a n", a=6)
        wv = pers.tile([128, 6 * DHC], BF16, tag="wv", name="wv")
        nc.sync.dma_start(wv[:], wv_d.rearrange("(a p) n -> p a n", p=128))
        wv3 = wv.rearrange("p (a n) -> p a n", a=6)
        t_ge = pers.tile([128, 128], BF16, tag="t_ge", name="t_ge")
        t_gl = pers.tile([128, 256], BF16, tag="t_gl", name="t_gl")
        t_le = pers.tile([128, 128], BF16, tag="t_le", name="t_le")
        nc.sync.dma_start(t_ge[:], tge_d)
        nc.sync.dma_start(t_gl[:], tgl_d)
        nc.sync.dma_start(t_le[:], tle_d)
        if use_b:
            bqk = pers.tile([128, 3], F32, tag="bqk", name="bqk")
            nc.sync.dma_start(bqk[:], bqk_d.rearrange("(m p) c -> p (m c)", p=128))
        if use_bv:
            bvr = pers.tile([1, DHC], BF16, tag="bvr", name="bvr")
            nc.sync.dma_start(bvr[:], bvr_d)
        if use_fmask:
            fmk = pers.tile([128, NKT], F32, tag="fmk", name="fmk")
            nc.sync.dma_start(fmk[:], fmk_d)
        if use_qmask:
            qmk = pers.tile([128, NKT], F32, tag="qmk", name="qmk")
            nc.sync.dma_start(qmk[:], qmk_d)

        # persistent activations
        xT = pers.tile([128, 6 * S], BF16, tag="xT", name="xT")
        xT3 = xT.rearrange("p (a s) -> p a s", a=6)
        # per-head zero-padded Q^T/K^T: head data rows 0-63, zeros 64-127
        qz = [pers.tile([128, S], BF16, tag=f"qz{h}", name=f"qz{h}")
              for h in range(HPC)]
        kz = [pers.tile([128, S], BF16, tag=f"kz{h}", name=f"kz{h}")
              for h in range(HPC)]
        # staging for the projection outputs that land on PSUM rows 64-127
        # (q-h1, k-h1, k-h2); moved to rows 0-63 of qz/kz by SBUF->SBUF DMA
        stg = [pers.tile([128, S], BF16, tag=f"stg{i}", name=f"stg{i}")
               for i in range(3)]
        for t in qz + kz:
            nc.gpsimd.memset(t[64:128, :], 0.0)
        va = pers.tile([128, NKT * HPC * VAW], BF16, tag="va", name="va")
        va4 = va.rearrange("p (t h c) -> p t h c", h=HPC, c=VAW)
        nc.gpsimd.memset(va4[:, :, :, DH:VAW], 1.0)
        # pre-warm the exp table while phase 1 runs
        scr = pers.tile([1, 1], BF16, tag="scr", name="scr")
        nc.scalar.activation(scr[:], t_ge[0:1, 0:1], ActFn.Exp)

        # ---------------- phase 1: X^T, projections, V_aug ----------------
        with tc.tile_pool(name="psB", bufs=2, space="PSUM") as psB, \
             tc.tile_pool(name="psC", bufs=4, space="PSUM") as psC:
            for sb in range(NSB):
                ssl = slice(sb * SBLK, (sb + 1) * SBLK)
                if sb % 2 == 0:
                    # one xbar call per (dt, 1024-row slab): amortizes the
                    # ~700ns per-call DGE overhead over twice the tiles
                    psl = slice(sb * SBLK, (sb + 2) * SBLK)
                    for dt in range(6):
                        nc.sync.dma_start_transpose(
                            xT3[:, dt, psl],
                            xb_d[psl, dt * 128:(dt + 1) * 128])
                # fused Q/K projection: m0 = q h0 | q h1, m1 = k h0 | k h1,
                # m2 = q h2 | k h2
                for m in range(3):
                    ps = psB.tile([128, SBLK], F32, tag="pb", name=f"pb{m}_{sb}",
                                  bufs=2)
                    for k in range(6):
                        nc.tensor.matmul(
                            ps[:],
                            wqk3[:, k, m * 128:(m + 1) * 128],
                            xT3[:, k, ssl],
                            start=(k == 0), stop=(k == 5))
                    if m == 0:
                        dsts = [(qz[0][0:64, ssl], ps[0:64, :], (0, 64)),
                                (stg[0][64:128, ssl], ps[64:128, :], (64, 128))]
                    elif m == 1:
                        dsts = [(kz[0][0:64, ssl], ps[0:64, :], (0, 64)),
                                (stg[1][64:128, ssl], ps[64:128, :], (64, 128))]
                    else:
                        dsts = [(qz[2][0:64, ssl], ps[0:64, :], (0, 64)),
                                (stg[2][64:128, ssl], ps[64:128, :], (64, 128))]
                    for dst, src, rows in dsts:
                        if use_b:
                            bias = bqk[rows[0]:rows[1], m:m + 1]
                            nc.vector.tensor_scalar_add(dst, src, bias)
                        else:
                            nc.vector.tensor_copy(dst, src)
                # V row-major into V_aug
                for st in range(sb * 4, sb * 4 + 4):
                    vps = psC.tile([128, DHC], F32, tag="vps", name="vps",
                                   bufs=4)
                    for k in range(6):
                        nc.tensor.matmul(
                            vps[:],
                            xT3[:, k, st * 128:(st + 1) * 128],
                            wv3[:, k, :],
                            start=(k == 0), stop=(k == 5 and not use_bv))
                    if use_bv:
                        # bias via K=1 ones-row matmul (t_ge row 127 = ones)
                        nc.tensor.matmul(
                            vps[:], t_ge[127:128, 0:128], bvr[0:1, :],
                            start=False, stop=True)
                    nc.vector.tensor_copy(
                        va4[:, st, :, 0:DH],
                        vps.rearrange("p (h d) -> p h d", h=HPC))
            # move staged halves down to rows 0-63 of their padded tiles
            nc.sync.dma_start(qz[1][0:64, :], stg[0][64:128, :])
            nc.sync.dma_start(kz[1][0:64, :], stg[1][64:128, :])
            nc.sync.dma_start(kz[2][0:64, :], stg[2][64:128, :])

        # ---------------- phase 2: banded attention ----------------
        # slot layout per (chunk, head): slot 0 cols [0:128] = edge tile
        # j=-2 half0 (t_ge), cols [128:256] = edge tile j=3 half1 (t_le);
        # slots 1..4 = full tiles j=-1..2.
        def slots_for(ci):
            """[(slot, col0, ncol, kt, half_or_None)]"""
            out = []
            kt_lo = 2 * ci - 2
            if kt_lo >= 0:
                out.append((0, 0, 128, kt_lo, 0))
            kt_hi = 2 * ci + 3
            if kt_hi <= NKT - 1:
                out.append((0, 128, 128, kt_hi, 1))
            for j in range(-1, 3):
                kt = 2 * ci + j
                if 0 <= kt <= NKT - 1:
                    out.append((1 + j + 1, (j + 2) * C2, C2, kt, None))
            return out

        with tc.tile_pool(name="pD", bufs=1) as pD, \
             tc.tile_pool(name="psD_sc", bufs=2, space="PSUM") as psD_sc, \
             tc.tile_pool(name="psD_av", bufs=2, space="PSUM") as psD_av:
            state = {}

            def front(ci):
                qsl = slice(ci * C2, (ci + 1) * C2)
                sl = slots_for(ci)
                scs = [psD_sc.tile([128, SCW], F32, tag="sc",
                                   name=f"sc{ci}_{h}", bufs=2)
                       for h in range(HPC)]
                for _, c0, nc_, kt, hf in sl:
                    ksl = slice(kt * 128, (kt + 1) * 128)
                    if hf is None:
                        q0 = qsl
                    else:
                        q0 = slice(ci * C2 + hf * 128, ci * C2 + hf * 128 + 128)
                    osl = slice(c0, c0 + nc_)
                    for h in range(HPC):
                        nc.tensor.matmul(scs[h][:, osl], kz[h][:, ksl],
                                         qz[h][:, q0], start=True, stop=True)
                hi = SCW if 2 * ci + 2 <= NKT - 1 else 4 * C2
                pts = []
                for h in range(HPC):
                    pt = pD.tile([128, SCW], BF16, tag="pt",
                                 name=f"pt{ci}_{h}", bufs=6)
                    nc.scalar.activation(pt[:, 0:hi], scs[h][:, 0:hi],
                                         ActFn.Exp)
                    pts.append(pt)
                state[ci] = pts

            def mid(ci):
                pts = state[ci]
                sl = slots_for(ci)
                has_lo = 2 * ci - 2 >= 0
                has_hi = 2 * ci + 3 <= NKT - 1
                av_big = psD_av.tile([128, 6 * VAW], F32, tag="av",
                                     name=f"av{ci}", bufs=2)
                av6 = av_big.rearrange("p (g c) -> p g c", c=VAW)
                state[(ci, 'av')] = av6
                for h in range(HPC):
                    pt = pts[h]
                    if has_lo and has_hi:
                        nc.vector.tensor_tensor(pt[:, 0:256], pt[:, 0:256],
                                                t_gl[:], op=AluOp.mult)
                    elif has_lo:
                        nc.vector.tensor_tensor(pt[:, 0:128], pt[:, 0:128],
                                                t_ge[:], op=AluOp.mult)
                    elif has_hi:
                        nc.vector.tensor_tensor(pt[:, 128:256], pt[:, 128:256],
                                                t_le[:], op=AluOp.mult)
                    # full-tile edge masks: j=-1 half1 (t_ge), j=2 half0 (t_le)
                    for slot, c0, nc_, kt, hf in sl:
                        j = kt - 2 * ci
                        if hf is not None:
                            continue
                        if j == -1:
                            nc.vector.tensor_tensor(
                                pt[:, c0 + 128:c0 + 256],
                                pt[:, c0 + 128:c0 + 256], t_ge[:],
                                op=AluOp.mult)
                        elif j == 2:
                            nc.vector.tensor_tensor(
                                pt[:, c0:c0 + 128], pt[:, c0:c0 + 128],
                                t_le[:], op=AluOp.mult)
                    if use_fmask:
                        for slot, c0, nc_, kt, hf in sl:
                            nc.vector.tensor_scalar_mul(
                                pt[:, c0:c0 + nc_], pt[:, c0:c0 + nc_],
                                fmk[:, kt:kt + 1])
                    # AV accumulation
                    for hf in range(2):
                        lst = []
                        for slot, c0, nc_, kt, shf in sl:
                            if shf is None:
                                lst.append((kt, pt[:, c0 + hf * 128:
                                                   c0 + hf * 128 + 128]))
                            elif shf == hf:
                                lst.append((kt, pt[:, c0:c0 + 128]))
                        lst.sort(key=lambda x: x[0])
                        g = h * 2 + hf
                        for i, (kt, psl) in enumerate(lst):
                            nc.tensor.matmul(
                                av6[:, g, :], psl, va4[:, kt, h, :],
                                start=(i == 0), stop=(i == len(lst) - 1))

            def tail(ci):
                av6 = state.pop((ci, 'av'))
                state.pop(ci)
                os_t = [pD.tile([128, DHC], F32, tag="os", name=f"os{ci}_{hf}",
                                bufs=4) for hf in range(2)]
                rzs = pD.tile([128, 6], F32, tag="rzs", name=f"rzs{ci}",
                              bufs=3)
                nc.vector.reciprocal(rzs[:], av6[:, :, DH])
                if use_qmask:
                    for g in range(6):
                        nc.vector.tensor_scalar_mul(
                            rzs[:, g:g + 1], rzs[:, g:g + 1],
                            qmk[:, 2 * ci + (g % 2):2 * ci + (g % 2) + 1])
                for h in range(HPC):
                    for hf in range(2):
                        g = h * 2 + hf
                        nc.vector.tensor_scalar_mul(
                            os_t[hf][:, h * DH:(h + 1) * DH],
                            av6[:, g, 0:DH], rzs[:, g:g + 1])
                for hf in range(2):
                    qt = 2 * ci + hf
                    nc.sync.dma_start(
                        out_d[qt * 128:(qt + 1) * 128, :], os_t[hf][:])

            for ci in range(NCH):
                front(ci)
                if ci >= 1:
                    mid(ci - 1)
                if ci >= 2:
                    tail(ci - 2)
            mid(NCH - 1)
            tail(NCH - 2)
            tail(NCH - 1)

    nc.compile()
    return nc


_prog_cache = {}


def _get_program(use_b, use_bv, use_fmask, use_qmask):
    key = (use_b, use_bv, use_fmask, use_qmask)
    if key not in _prog_cache:
        _prog_cache[key] = _build_program(use_b, use_bv, use_fmask, use_qmask)
    return _prog_cache[key]


def _host_constants():
    kl = np.arange(128)[:, None]
    ql = np.arange(128)[None, :]
    t_ge = (kl >= ql).astype(NPBF16)
    t_le = (kl <= ql).astype(NPBF16)
    t_gl = np.concatenate([t_ge, t_le], axis=1)
    return t_ge, t_gl, t_le


def kernel(hidden_states, attention_mask, is_index_masked, Wq, bq, Wk, bk, Wv, bv,
           trace=False):
    hidden_states = np.asarray(hidden_states, dtype=np.float32)
    attention_mask = np.asarray(attention_mask, dtype=np.float32)
    is_index_masked = np.asarray(is_index_masked)
    Wq = np.asarray(Wq, dtype=np.float32)
    Wk = np.asarray(Wk, dtype=np.float32)
    Wv = np.asarray(Wv, dtype=np.float32)
    bq = np.asarray(bq, dtype=np.float32)
    bk = np.asarray(bk, dtype=np.float32)
    bv = np.asarray(bv, dtype=np.float32)

    use_b = bool(np.any(bq != 0) or np.any(bk != 0))
    use_bv = bool(np.any(bv != 0))
    use_fmask = bool(np.any(attention_mask != 0))
    use_qmask = bool(np.any(is_index_masked))
    nc = _get_program(use_b, use_bv, use_fmask, use_qmask)

    scale = 1.0 / math.sqrt(DH)
    t_ge, t_gl, t_le = _host_constants()

    xb = hidden_states.astype(NPBF16)

    in_maps = []
    for cid in range(NCORES):
        b = cid // 4
        h0 = HPC * (cid % 4)
        c0, c1 = h0 * DH, (h0 + HPC) * DH
        wql = Wq[:, c0:c1] * scale
        wkl = Wk[:, c0:c1]
        wqk_h = np.concatenate(
            [wql[:, 0:128], wkl[:, 0:128], wql[:, 128:192], wkl[:, 128:192]],
            axis=1).astype(NPBF16)
        m = {
            "xb": xb[b],
            "wqk": np.ascontiguousarray(wqk_h),
            "wv": np.ascontiguousarray(Wv[:, c0:c1].astype(NPBF16)),
            "t_ge": t_ge,
            "t_gl": t_gl,
            "t_le": t_le,
        }
        if use_b:
            bql = bq[c0:c1] * scale
            bkl = bk[c0:c1]
            m["bqk"] = np.ascontiguousarray(np.concatenate(
                [bql[0:128], bkl[0:128], bql[128:192], bkl[128:192]])
                .reshape(MQK, 1))
        if use_bv:
            m["bvr"] = np.ascontiguousarray(
                bv[c0:c1].astype(NPBF16).reshape(1, DHC))
        if use_fmask:
            fac = (attention_mask[b] == 0).astype(np.float32)
            m["fmk"] = np.ascontiguousarray(fac.reshape(NKT, 128).T)
        if use_qmask:
            keep = (~is_index_masked[b]).astype(np.float32)
            m["qmk"] = np.ascontiguousarray(keep.reshape(NKT, 128).T)
        in_maps.append(m)

    res = run_bass_kernel_spmd(nc, in_maps, core_ids=list(range(NCORES)),
                               trace=trace)
    out = np.empty((B, S, D), dtype=np.float32)
    for cid in range(NCORES):
        b = cid // 4
        h0 = HPC * (cid % 4)
        out[b, :, h0 * DH:(h0 + HPC) * DH] = res.results[cid]["out"]
    if trace:
        return out, res
    return out
